# revision 11
# baseline (speedup 1.0000x reference)
"""DualPathRNN Trainium2 kernel.

12 sequential LSTM layers (C=256, T=4000) over B=16, data-parallel over batch
across 8 NeuronCores (2 batch elements per core). Everything per-layer runs on
one core:

  - input projection GEMM (W_ih @ x + biases) fused into the scan loop, ~50
    timesteps look-ahead, bf16 weights, fp32 psum; inputs staged via SBUF DMA
    so all matmul access patterns are static
  - the serial 4000-step LSTM scan: per step, W_hh (permuted into 16 bf16
    128x128 chunks) is loaded as the stationary operand (FWL) and multiplied
    against h(t-1) (bf16, N=2) read from a static-slot ring. Gate
    pre-activations accumulate into THREE psum groups (g / i,f / o) with
    separate stop flags so tanh(g) and sigmoid(i,f) start while the rest of
    the matmul block is still issuing; gx is injected per group via identity
    matmuls that run during the previous step's activation tail. The c update
    is a single fused VectorE tensor_tensor_scan over interleaved
    (0,sf)x(c,u) pairs (c' = sf*c + u); parity-alternating tiles keep
    write-after-read deps one step removed. h ring is DMA-copied to the full
    y buffer once per loop iteration.
  - residual + time-permutation (even layers) / time-flip (odd layers) as a
    handful of big strided VectorE ops between layers

Self-contained: hardcodes shapes from the problem spec.
"""
import os
import sys

sys.path.insert(0, "/opt/trn_rl_repo")

import numpy as np
import ml_dtypes

from concourse import bass, bacc, mybir
import concourse.tile as tile
from concourse.bass import ds
from concourse.bass_utils import run_bass_kernel_spmd

F32 = mybir.dt.float32
BF16 = mybir.dt.bfloat16
AF = mybir.ActivationFunctionType
ALU = mybir.AluOpType
ET = mybir.EngineType
BF = ml_dtypes.bfloat16

# Problem constants
C = 256
NL = 12
B = 16
L = 4000
IL = 10
NCORES = 8
BPC = B // NCORES  # 2 batch elements per core

# slot order within a 128-channel half: i, f, o, g ; ref row-gate order i,f,g,o
GMAP = [0, 1, 3, 2]


def _mkap(t, off, dims):
    """Build an AP on tile t's tensor: partition dim from t, free dims given as
    [(stride, count), ...] in elements; off is the element offset (may be a
    ScalarValue expression)."""
    base = t[:, 0:1]
    return bass.AP(
        tensor=base.tensor,
        offset=base.offset + off,
        ap=[list(base.ap[0])] + [[s, n] for (s, n) in dims],
    )


def build_kernel(nc, T=L, n_layers=NL, SUB=25, NSUB=4):
    U = SUB * NSUB
    NIT = T // U
    assert NIT * U == T
    NDL = n_layers // 2
    Tpad = T + 2 * SUB
    b = BPC

    x_in = nc.dram_tensor("x_in", [b, C, T], F32, kind="ExternalInput")
    whh_d = nc.dram_tensor("whh_all", [n_layers * 128, 2048], BF16, kind="ExternalInput")
    wih_d = nc.dram_tensor("wih_all", [n_layers * 128, 2048], BF16, kind="ExternalInput")
    bias_d = nc.dram_tensor("bias_all", [n_layers * 128, 16 * SUB], F32, kind="ExternalInput")
    ident_d = nc.dram_tensor("ident", [128, 128], F32, kind="ExternalInput")
    out_d = nc.dram_tensor("out", [b, C, T], F32, kind="ExternalOutput")

    with tile.TileContext(nc) as tc:
        with (
            tc.tile_pool(name="persist", bufs=1) as pp,
            tc.tile_pool(name="chain", bufs=6) as cp,
            tc.tile_pool(name="stage", bufs=3) as sp,
            tc.tile_pool(name="psStep", bufs=1, space="PSUM") as ppa,
            tc.tile_pool(name="psG", bufs=2, space="PSUM") as ppg,
        ):
            x32 = pp.tile([128, 4 * Tpad], F32, tag="x32")
            xb = pp.tile([128, 4 * Tpad], BF16, tag="xb")
            ybig = pp.tile([128, 4 * T], BF16, tag="ybig")
            ring = pp.tile([128, 4 * (U + 1)], BF16, tag="ring")
            ident = pp.tile([128, 128], F32, tag="ident")
            identb = pp.tile([128, 128], BF16, tag="identb")
            # sigmoid staging (parity pair to keep WAR deps ancient):
            # sigma(i) at cols {0,2,4,6}; cols 8..15 hold (0, sf0, 0, sf1, ..)
            # so [8:16] is the scan's d0 operand
            sigif = [pp.tile([128, 16], F32, tag=f"sigif{q}", name=f"sigif{q}")
                     for q in range(2)]
            # c/u pair tiles (ping-pong per step): c at odd cols {1,3,5,7},
            # u at even cols {2,4,6,8}
            cpair = [pp.tile([128, 9], F32, tag=f"cpair{q}", name=f"cpair{q}")
                     for q in range(2)]
            gtt = [pp.tile([128, 4], F32, tag=f"gt{q}", name=f"gt{q}")
                   for q in range(2)]
            tcht = [pp.tile([128, 4], F32, tag=f"tch{q}", name=f"tch{q}")
                    for q in range(2)]
            tmpr = pp.tile([128, T], F32, tag="tmpr")
            whh = [pp.tile([128, 2048], BF16, tag=f"whh{p}", name=f"whh{p}") for p in range(2)]
            wih = [pp.tile([128, 2048], BF16, tag=f"wih{p}", name=f"wih{p}") for p in range(2)]
            biasb = [pp.tile([128, 16 * SUB], F32, tag=f"bias{p}", name=f"bias{p}") for p in range(2)]
            gxr = [pp.tile([128, 16 * SUB], BF16, tag=f"gxr{q}", name=f"gxr{q}") for q in range(4)]


            # ---- prologue: load inputs, build fp32 + bf16 x images ----
            nc.sync.dma_start(ident[:, :], ident_d[:, :])
            nc.vector.tensor_copy(identb[:, :], ident[:, :])
            nc.vector.memset(sigif[0][:, :], 0.0)
            nc.vector.memset(sigif[1][:, :], 0.0)
            for hb in range(2):
                for beta in range(2):
                    seg = hb * 2 + beta
                    nc.sync.dma_start(
                        x32[:, seg * Tpad : seg * Tpad + T],
                        x_in[beta, hb * 128 : (hb + 1) * 128, :],
                    )
            for seg in range(4):
                nc.vector.memset(x32[:, seg * Tpad + T : (seg + 1) * Tpad], 0.0)
            for seg in range(4):
                nc.vector.tensor_copy(
                    xb[:, seg * Tpad : (seg + 1) * Tpad],
                    x32[:, seg * Tpad : (seg + 1) * Tpad],
                )

            def emit_gemm_tile(par, tg0, slot):
                """Compute gx for steps [tg0, tg0+SUB) into ring slot `slot`.
                tg0 may be a ScalarValue expression or int. All matmul APs are
                static; the x slice is staged via DMA."""
                stg = sp.tile([128, 4 * SUB], BF16, tag="stg", name="stg")
                # stage [seg][tau] <- xb[seg*Tpad + tg0 + tau]
                nc.sync.dma_start(
                    stg[:, :], _mkap(xb, tg0, [(Tpad, 4), (1, SUB)])
                )
                psG = ppg.tile([128, 16 * SUB], F32, tag="psG", name="psG")
                # bias inject (fp32): psG[m*2S + tr*2 + beta] = bias_bcast
                nc.tensor.matmul(
                    psG[:, :], ident[:, :], biasb[par][:, :], start=True, stop=False
                )
                for kc in range(2):
                    rhs = _mkap(stg, kc * 2 * SUB, [(1, SUB), (SUB, 2)])
                    for m in range(8):
                        nc.tensor.matmul(
                            psG[:, m * 2 * SUB : (m + 1) * 2 * SUB],
                            wih[par][:, (m * 2 + kc) * 128 : (m * 2 + kc + 1) * 128],
                            rhs,
                            start=False,
                            stop=(kc == 1 and m == 7),
                        )
                # reorder copy psum [m][tr][beta] -> ring [tr][slot=m][beta]
                src = _mkap(psG, 0, [(2 * SUB, 8), (2, SUB), (1, 2)])
                dst = _mkap(gxr[slot], 0, [(2, 8), (16, SUB), (1, 2)])
                nc.vector.tensor_copy(dst, src)

            def emit_step(par, off, gx_sl):
                """One LSTM step at in-body offset `off` (static). Reads h(t-1)
                from ring slot off, writes h(t) to ring slot off+1.
                Slot order in psum/gx: [g_lo,g_hi, i_lo,i_hi, f_lo,f_hi, o_lo,o_hi]
                x [beta]; free idx = slot*2+beta.

                Three separate psum accumulation groups (g / i,f / o) with
                their own stops so tanh(g) and sigmoid(i,f) overlap the tail
                of the matmul block. c-update is one fused tensor_tensor_scan:
                pairs (0, sf) x (c, u) -> c' = sf*c + u."""
                p = off % 2
                src, dst = cpair[p], cpair[1 - p]
                psG = ppa.tile([128, 4], F32, tag="pG", name="pG")
                psIF = ppa.tile([128, 8], F32, tag="pIF", name="pIF")
                psO = ppa.tile([128, 4], F32, tag="pO", name="pO")
                # gx injects (identity matmul), open the accum groups
                nc.tensor.matmul(
                    psG[:, :], identb[:, :], gx_sl[:, 0:4], start=True, stop=False
                )
                nc.tensor.matmul(
                    psIF[:, :], identb[:, :], gx_sl[:, 4:12], start=True, stop=False
                )
                nc.tensor.matmul(
                    psO[:, :], identb[:, :], gx_sl[:, 12:16], start=True, stop=False
                )
                # W_hh matmuls, slot-major (g,g,i,i,f,f,o,o) with per-group stop
                for s in range(8):
                    if s < 2:
                        ps, col0 = psG, s * 2
                    elif s < 6:
                        ps, col0 = psIF, (s - 2) * 2
                    else:
                        ps, col0 = psO, (s - 6) * 2
                    for kc in range(2):
                        rhs = ring[:, off * 4 + kc * 2 : off * 4 + kc * 2 + 2]
                        nc.tensor.matmul(
                            ps[:, col0 : col0 + 2],
                            whh[par][:, (s * 2 + kc) * 128 : (s * 2 + kc + 1) * 128],
                            rhs,
                            start=False,
                            stop=(kc == 1 and s in (1, 5, 7)),
                        )
                # chain: tanh_g + sigmoid(i,f) overlap the MM tail; scan c-update
                gt = gtt[p]
                tch = tcht[p]
                sgf = sigif[p]
                sigo = cp.tile([128, 4], F32, tag="sigo", name="sigo")
                nc.scalar.activation(gt[:, :], psG[:, :], AF.Tanh)
                nc.scalar.activation(
                    _mkap(sgf, 0, [(9, 2), (2, 4)]),
                    _mkap(psIF, 0, [(4, 2), (1, 4)]),
                    AF.Sigmoid,
                )
                nc.scalar.activation(sigo[:, :], psO[:, :], AF.Sigmoid)
                # u = sig_i * tanh_g -> src even cols {2,4,6,8}
                nc.vector.tensor_mul(
                    _mkap(src, 2, [(2, 4)]), _mkap(sgf, 0, [(2, 4)]), gt[:, :]
                )
                # c' = sf*c + u via scan over (0,sf0,0,sf1,..) x (c0,u0,c1,u1,..)
                nc.vector.tensor_tensor_scan(
                    dst[:, 0:8], sgf[:, 8:16], src[:, 1:9], 0.0,
                    ALU.mult, ALU.add,
                )
                nc.scalar.activation(tch[:, :], _mkap(dst, 1, [(2, 4)]), AF.Tanh)
                nc.vector.tensor_mul(
                    ring[:, (off + 1) * 4 : (off + 2) * 4], sigo[:, :], tch[:, :]
                )

            def emit_scan(par):
                # init state
                nc.vector.memset(ring[:, 0:4], 0.0)
                nc.vector.memset(cpair[0][:, :], 0.0)
                nc.vector.memset(cpair[1][:, :], 0.0)
                # prime gx ring slots 0,1 (steps 0..2*SUB)
                for q in range(2):
                    emit_gemm_tile(par, q * SUB, q)
                with tc.For_i(0, NIT, 1) as it:
                    tg = it * U
                    for q in range(NSUB):
                        for tr in range(SUB):
                            off = q * SUB + tr
                            emit_step(par, off, gxr[q % 4][:, tr * 16 : (tr + 1) * 16])
                        emit_gemm_tile(par, tg + (q + 2) * SUB, (q + 2) % 4)
                    # drain h ring to the big y buffer; wrap last h to slot 0
                    t4 = it * (4 * U)
                    nc.sync.dma_start(
                        ybig[:, ds(t4, 4 * U)], ring[:, 4 : 4 * (U + 1)]
                    )
                    nc.vector.tensor_copy(
                        ring[:, 0:4], ring[:, 4 * U : 4 * (U + 1)]
                    )

            def emit_residual(par):
                if par == 0:
                    # x[t'] += y[i*(T/IL)+j] for t' = j*IL + i  (in-place)
                    for hb in range(2):
                        for beta in range(2):
                            seg = hb * 2 + beta
                            xap = _mkap(x32, seg * Tpad, [(IL, T // IL), (1, IL)])
                            xap2 = _mkap(x32, seg * Tpad, [(IL, T // IL), (1, IL)])
                            yap = _mkap(
                                ybig, hb * 2 + beta,
                                [(4, T // IL), (4 * (T // IL), IL)],
                            )
                            nc.vector.tensor_tensor(xap, xap2, yap, ALU.add)
                else:
                    # x_new[t'] = x[T-1-t'] + y[T-1-t']  (flip, via tmp)
                    for hb in range(2):
                        for beta in range(2):
                            seg = hb * 2 + beta
                            yap = _mkap(ybig, hb * 2 + beta, [(4, T)])
                            nc.vector.tensor_tensor(
                                tmpr[:, :],
                                x32[:, seg * Tpad : seg * Tpad + T],
                                yap,
                                ALU.add,
                            )
                            rev = _mkap(tmpr, T - 1, [(-1, T)])
                            nc.vector.tensor_copy(
                                x32[:, seg * Tpad : seg * Tpad + T], rev
                            )
                # refresh bf16 image
                for seg in range(4):
                    nc.vector.tensor_copy(
                        xb[:, seg * Tpad : seg * Tpad + T],
                        x32[:, seg * Tpad : seg * Tpad + T],
                    )

            # ---- layer loop: 2 layers (even, odd) per iteration ----
            with tc.For_i(0, NDL, 1) as lj:
                for par in range(2):
                    lidx = lj * 2 + par
                    nc.sync.dma_start(whh[par][:, :], whh_d[ds(lidx * 128, 128), :])
                    nc.sync.dma_start(wih[par][:, :], wih_d[ds(lidx * 128, 128), :])
                    nc.sync.dma_start(biasb[par][:, :], bias_d[ds(lidx * 128, 128), :])
                    emit_scan(par)
                    emit_residual(par)

            # ---- epilogue: store ----
            for hb in range(2):
                for beta in range(2):
                    seg = hb * 2 + beta
                    nc.sync.dma_start(
                        out_d[beta, hb * 128 : (hb + 1) * 128, :],
                        x32[:, seg * Tpad : seg * Tpad + T],
                    )
    return nc


def prep_weights(w_ih, w_hh, b_ih, b_hh, n_layers, SUB=25):
    """Permute/transpose weights into the SBUF chunk layouts (host side)."""
    whh_all = np.zeros((n_layers * 128, 2048), BF)
    wih_all = np.zeros((n_layers * 128, 2048), BF)
    bias_all = np.zeros((n_layers * 128, 16 * SUB), np.float32)
    # slot order [g_lo,g_hi, i_lo,i_hi, f_lo,f_hi, o_lo,o_hi]; ref gates i,f,g,o
    SLOTS = [(2, 0), (2, 1), (0, 0), (0, 1), (1, 0), (1, 1), (3, 0), (3, 1)]
    for k in range(n_layers):
        bias = (b_ih[k] + b_hh[k]).astype(np.float32)
        for s in range(8):
            g, hf = SLOTS[s]
            r0 = g * C + hf * 128
            rows_hh = w_hh[k][r0 : r0 + 128]  # (128, 256)
            rows_ih = w_ih[k][r0 : r0 + 128]
            for kc in range(2):
                col = (s * 2 + kc) * 128
                whh_all[k * 128 : (k + 1) * 128, col : col + 128] = (
                    rows_hh[:, kc * 128 : (kc + 1) * 128].T.astype(BF)
                )
                wih_all[k * 128 : (k + 1) * 128, col : col + 128] = (
                    rows_ih[:, kc * 128 : (kc + 1) * 128].T.astype(BF)
                )
            # bias layout [m][tr][beta], m == slot
            bb = bias[r0 : r0 + 128]  # (128,)
            bias_all[k * 128 : (k + 1) * 128, s * 2 * SUB : (s + 1) * 2 * SUB] = (
                np.repeat(bb[:, None], 2 * SUB, axis=1)
            )
    return whh_all, wih_all, bias_all


def _timed_pjrt_run(nc, in_maps, n_timing=3):
    """Compile once via PJRT, run repeatedly on the 8 cores, return
    (per-core results, best wall-clock ns per execution)."""
    import time as _time

    import jax
    from jax.sharding import Mesh, PartitionSpec, NamedSharding
    from jax.experimental.shard_map import shard_map

    from concourse import bass2jax, mybir as _mybir

    bass2jax.install_neuronx_cc_hook()
    n_cores = len(in_maps)

    partition_name = nc.partition_id_tensor.name if nc.partition_id_tensor else None
    in_names, out_names, out_avals, zero_outs = [], [], [], []
    for alloc in nc.m.functions[0].allocations:
        if not isinstance(alloc, _mybir.MemoryLocationSet):
            continue
        name = alloc.memorylocations[0].name
        if alloc.kind == "ExternalInput":
            if name != partition_name:
                in_names.append(name)
        elif alloc.kind == "ExternalOutput":
            shape = tuple(alloc.tensor_shape)
            dtype = _mybir.dt.np(alloc.dtype)
            out_names.append(name)
            out_avals.append(jax.core.ShapedArray(shape, dtype))
            zero_outs.append(np.zeros(shape, dtype))
    n_params = len(in_names)
    all_in_names = list(in_names) + list(out_names)
    if partition_name is not None:
        all_in_names.append(partition_name)

    def _body(*args):
        operands = list(args)
        if partition_name is not None:
            operands.append(bass2jax.partition_id_tensor())
        outs = bass2jax._bass_exec_p.bind(
                *operands,
                out_avals=tuple(out_avals),
                in_names=tuple(all_in_names),
                out_names=tuple(out_names),
                lowering_input_output_aliases=(),
                sim_require_finite=True,
                sim_require_nnan=True,
                nc=nc,
            )
        return tuple(outs)

    devices = jax.devices()[:n_cores]
    mesh = Mesh(np.asarray(devices), ("core",))
    nsh = NamedSharding(mesh, PartitionSpec("core"))
    in_specs = (PartitionSpec("core"),) * (n_params + len(out_names))
    out_specs = (PartitionSpec("core"),) * len(out_names)
    sharded = jax.jit(
        shard_map(_body, mesh=mesh, in_specs=in_specs, out_specs=out_specs,
                  check_rep=False),
        keep_unused=True,
    )
    concat_in = [
        np.concatenate([np.asarray(in_maps[c][nm]) for c in range(n_cores)], axis=0)
        for nm in in_names
    ]
    concat_zeros = [
        np.zeros((n_cores * z.shape[0], *z.shape[1:]), z.dtype) for z in zero_outs
    ]
    dev_args = [jax.device_put(a, nsh) for a in concat_in + concat_zeros]
    outs = sharded(*dev_args)
    jax.block_until_ready(outs)
    best = None
    for _ in range(n_timing):
        t0 = _time.perf_counter()
        outs = sharded(*dev_args)
        jax.block_until_ready(outs)
        dt = (_time.perf_counter() - t0) * 1e9
        best = dt if best is None else min(best, dt)
    results = [
        {
            nm: np.asarray(outs[i]).reshape(n_cores, *out_avals[i].shape)[c]
            for i, nm in enumerate(out_names)
        }
        for c in range(n_cores)
    ]
    return results, best


def run(inputs, trace=False, T=None, n_layers=None, SUB=25, NSUB=8, n_timing=3):
    """Build+run with timing; returns (full output, best_exec_ns)."""
    return _kernel_impl(
        inputs["x"], inputs["w_ih"], inputs["w_hh"], inputs["b_ih"],
        inputs["b_hh"], T=T, n_layers=n_layers, SUB=SUB, NSUB=NSUB,
        timed=True, n_timing=n_timing,
    )


def kernel(x, w_ih, w_hh, b_ih, b_hh):
    out, _ = _kernel_impl(x, w_ih, w_hh, b_ih, b_hh, NSUB=8)
    return out


def _kernel_impl(x, w_ih, w_hh, b_ih, b_hh, T=None, n_layers=None, SUB=25,
                 NSUB=4, timed=False, n_timing=3):
    x = np.asarray(x, np.float32)
    w_ih = np.asarray(w_ih, np.float32)
    w_hh = np.asarray(w_hh, np.float32)
    b_ih = np.asarray(b_ih, np.float32)
    b_hh = np.asarray(b_hh, np.float32)
    Bb, Cc, Ll = x.shape
    if T is None:
        T = Ll
    if n_layers is None:
        n_layers = w_ih.shape[0]

    whh_all, wih_all, bias_all = prep_weights(w_ih, w_hh, b_ih, b_hh, n_layers, SUB)
    ident = np.eye(128, dtype=np.float32)

    nc = bacc.Bacc("TRN2", debug=False, target_bir_lowering=False, num_devices=NCORES)
    build_kernel(nc, T=T, n_layers=n_layers, SUB=SUB, NSUB=NSUB)
    nc.finalize()

    in_maps = []
    for core in range(NCORES):
        in_maps.append(
            {
                "x_in": x[core * BPC : (core + 1) * BPC, :, :T].copy(),
                "whh_all": whh_all,
                "wih_all": wih_all,
                "bias_all": bias_all,
                "ident": ident,
            }
        )
    if timed:
        results, best_ns = _timed_pjrt_run(nc, in_maps, n_timing=n_timing)
    else:
        res = run_bass_kernel_spmd(nc, in_maps, core_ids=list(range(NCORES)))
        results, best_ns = res.results, None
    out = np.concatenate([results[c]["out"] for c in range(NCORES)], axis=0)
    return out.astype(np.float32), best_ns


if __name__ == "__main__":
    # tiny smoke test vs golden numpy model
    rng = np.random.default_rng(0)
    T = int(os.environ.get("T", "200"))
    NLY = int(os.environ.get("NLY", "2"))
    SUBv = int(os.environ.get("SUBV", "25"))
    NSUBv = int(os.environ.get("NSUBV", "4"))
    x = rng.standard_normal((B, C, T), dtype=np.float32)
    k = 1.0 / np.sqrt(C)
    w_ih = rng.uniform(-k, k, (NL, 4 * C, C)).astype(np.float32)
    w_hh = rng.uniform(-k, k, (NL, 4 * C, C)).astype(np.float32)
    b_ih = rng.uniform(-k, k, (NL, 4 * C)).astype(np.float32)
    b_hh = rng.uniform(-k, k, (NL, 4 * C)).astype(np.float32)

    got, _ = _kernel_impl(
        x, w_ih[:NLY], w_hh[:NLY], b_ih[:NLY], b_hh[:NLY],
        T=T, n_layers=NLY, SUB=SUBv, NSUB=NSUBv,
    )

    from golden import run_golden

    exp = run_golden(x, w_ih[:NLY], w_hh[:NLY], b_ih[:NLY], b_hh[:NLY], NLY)
    err = np.linalg.norm(got - exp) / np.linalg.norm(exp)
    print(f"T={T} NLY={NLY} rel_l2 vs golden = {err:.3e}")



# revision 15
# speedup vs baseline: 1.0477x; 1.0477x over previous
"""DualPathRNN Trainium2 kernel.

12 sequential LSTM layers (C=256, T=4000) over B=16, data-parallel over batch
across 8 NeuronCores (2 batch elements per core). Everything per-layer runs on
one core:

  - input projection GEMM (W_ih @ x + biases) fused into the scan loop, ~50
    timesteps look-ahead, bf16 weights, fp32 psum; inputs staged via SBUF DMA
    so all matmul access patterns are static
  - the serial 4000-step LSTM scan: per step, W_hh (permuted into 16 bf16
    128x128 chunks) is loaded as the stationary operand (FWL) and multiplied
    against h(t-1) (bf16, N=2) read from a static-slot ring. Gate
    pre-activations accumulate into THREE psum groups (g / i,f / o) with
    separate stop flags so tanh(g) and sigmoid(i,f) start while the rest of
    the matmul block is still issuing; gx is injected per group via identity
    matmuls that run during the previous step's activation tail. The c update
    is a single fused VectorE tensor_tensor_scan over interleaved
    (0,sf)x(c,u) pairs (c' = sf*c + u); parity-alternating tiles keep
    write-after-read deps one step removed. h ring is DMA-copied to the full
    y buffer once per loop iteration.
  - residual + time-permutation (even layers) / time-flip (odd layers) as a
    handful of big strided VectorE ops between layers

Self-contained: hardcodes shapes from the problem spec.
"""
import os
import sys

sys.path.insert(0, "/opt/trn_rl_repo")

import numpy as np
import ml_dtypes

from concourse import bass, bacc, mybir
import concourse.tile as tile
from concourse.bass import ds
from concourse.bass_utils import run_bass_kernel_spmd

F32 = mybir.dt.float32
BF16 = mybir.dt.bfloat16
AF = mybir.ActivationFunctionType
ALU = mybir.AluOpType
ET = mybir.EngineType
BF = ml_dtypes.bfloat16

# Problem constants
C = 256
NL = 12
B = 16
L = 4000
IL = 10
NCORES = 8
BPC = B // NCORES  # 2 batch elements per core

# slot order within a 128-channel half: i, f, o, g ; ref row-gate order i,f,g,o
GMAP = [0, 1, 3, 2]


def _mkap(t, off, dims):
    """Build an AP on tile t's tensor: partition dim from t, free dims given as
    [(stride, count), ...] in elements; off is the element offset (may be a
    ScalarValue expression)."""
    base = t[:, 0:1]
    return bass.AP(
        tensor=base.tensor,
        offset=base.offset + off,
        ap=[list(base.ap[0])] + [[s, n] for (s, n) in dims],
    )


def build_kernel(nc, T=L, n_layers=NL, SUB=25, NSUB=4):
    U = SUB * NSUB
    NIT = T // U
    assert NIT * U == T
    NDL = n_layers // 2
    Tpad = T + 2 * SUB
    b = BPC

    x_in = nc.dram_tensor("x_in", [b, C, T], F32, kind="ExternalInput")
    whh_d = nc.dram_tensor("whh_all", [n_layers * 128, 2048], BF16, kind="ExternalInput")
    wih_d = nc.dram_tensor("wih_all", [n_layers * 128, 2048], BF16, kind="ExternalInput")
    bias_d = nc.dram_tensor("bias_all", [n_layers * 128, 16 * SUB], F32, kind="ExternalInput")
    ident_d = nc.dram_tensor("ident", [128, 128], F32, kind="ExternalInput")
    out_d = nc.dram_tensor("out", [b, C, T], F32, kind="ExternalOutput")

    with tile.TileContext(nc) as tc:
        with (
            tc.tile_pool(name="persist", bufs=1) as pp,
            tc.tile_pool(name="chain", bufs=6) as cp,
            tc.tile_pool(name="stage", bufs=3) as sp,
            tc.tile_pool(name="psStep", bufs=1, space="PSUM") as ppa,
            tc.tile_pool(name="psG", bufs=2, space="PSUM") as ppg,
        ):
            x32 = pp.tile([128, 4 * Tpad], F32, tag="x32")
            xb = pp.tile([128, 4 * Tpad], BF16, tag="xb")
            ybig = pp.tile([128, 4 * T], BF16, tag="ybig")
            ring = pp.tile([128, 4 * (U + 1)], BF16, tag="ring")
            ident = pp.tile([128, 128], F32, tag="ident")
            identb = pp.tile([128, 128], BF16, tag="identb")
            # sigmoid staging (parity pair to keep WAR deps ancient):
            # sigma(i) at cols {0,2,4,6}; cols 8..15 hold (0, sf0, 0, sf1, ..)
            # so [8:16] is the scan's d0 operand
            sigif = [pp.tile([128, 16], F32, tag=f"sigif{q}", name=f"sigif{q}")
                     for q in range(2)]
            # c/u pair tiles (ping-pong per step): c at odd cols {1,3,5,7},
            # u at even cols {2,4,6,8}
            cpair = [pp.tile([128, 9], F32, tag=f"cpair{q}", name=f"cpair{q}")
                     for q in range(2)]
            gtt = [pp.tile([128, 4], F32, tag=f"gt{q}", name=f"gt{q}")
                   for q in range(2)]
            tcht = [pp.tile([128, 4], F32, tag=f"tch{q}", name=f"tch{q}")
                    for q in range(2)]
            tmpr = pp.tile([128, T], F32, tag="tmpr")
            whh = [pp.tile([128, 2048], BF16, tag=f"whh{p}", name=f"whh{p}") for p in range(2)]
            wih = [pp.tile([128, 2048], BF16, tag=f"wih{p}", name=f"wih{p}") for p in range(2)]
            biasb = [pp.tile([128, 16 * SUB], F32, tag=f"bias{p}", name=f"bias{p}") for p in range(2)]
            gxr = [pp.tile([128, 16 * SUB], BF16, tag=f"gxr{q}", name=f"gxr{q}") for q in range(4)]


            # ---- prologue: load inputs, build fp32 + bf16 x images ----
            nc.sync.dma_start(ident[:, :], ident_d[:, :])
            nc.vector.tensor_copy(identb[:, :], ident[:, :])
            nc.vector.memset(sigif[0][:, :], 0.0)
            nc.vector.memset(sigif[1][:, :], 0.0)
            for hb in range(2):
                for beta in range(2):
                    seg = hb * 2 + beta
                    nc.sync.dma_start(
                        x32[:, seg * Tpad : seg * Tpad + T],
                        x_in[beta, hb * 128 : (hb + 1) * 128, :],
                    )
            for seg in range(4):
                nc.vector.memset(x32[:, seg * Tpad + T : (seg + 1) * Tpad], 0.0)
            for seg in range(4):
                nc.vector.tensor_copy(
                    xb[:, seg * Tpad : (seg + 1) * Tpad],
                    x32[:, seg * Tpad : (seg + 1) * Tpad],
                )

            def emit_gemm_tile(par, tg0, slot):
                """Compute gx for steps [tg0, tg0+SUB) into ring slot `slot`.
                tg0 may be a ScalarValue expression or int. All matmul APs are
                static; the x slice is staged via DMA."""
                stg = sp.tile([128, 4 * SUB], BF16, tag="stg", name="stg")
                # stage [seg][tau] <- xb[seg*Tpad + tg0 + tau]
                nc.sync.dma_start(
                    stg[:, :], _mkap(xb, tg0, [(Tpad, 4), (1, SUB)])
                )
                psG = ppg.tile([128, 16 * SUB], F32, tag="psG", name="psG")
                # zero the bank on DVE, then accumulate W_ih matmuls with no
                # start flag (keeps the slow fp32 N=400 bias-inject matmul off
                # the PE queue; accumulate-onto-zeros is overwrite-equivalent)
                nc.vector.memset(psG[:, :], 0.0)
                for kc in range(2):
                    rhs = _mkap(stg, kc * 2 * SUB, [(1, SUB), (SUB, 2)])
                    for m in range(8):
                        nc.tensor.matmul(
                            psG[:, m * 2 * SUB : (m + 1) * 2 * SUB],
                            wih[par][:, (m * 2 + kc) * 128 : (m * 2 + kc + 1) * 128],
                            rhs,
                            start=False,
                            stop=(kc == 1 and m == 7),
                        )
                # bias add (contiguous, layouts match), then reorder copy
                # psum [m][tr][beta] -> ring [tr][slot=m][beta]
                nc.vector.tensor_tensor(
                    psG[:, :], psG[:, :], biasb[par][:, :], ALU.add
                )
                src = _mkap(psG, 0, [(2 * SUB, 8), (2, SUB), (1, 2)])
                dst = _mkap(gxr[slot], 0, [(2, 8), (16, SUB), (1, 2)])
                nc.vector.tensor_copy(dst, src)

            def emit_step(par, off, gx_sl):
                """One LSTM step at in-body offset `off` (static). Reads h(t-1)
                from ring slot off, writes h(t) to ring slot off+1.
                Slot order in psum/gx: [g_lo,g_hi, i_lo,i_hi, f_lo,f_hi, o_lo,o_hi]
                x [beta]; free idx = slot*2+beta.

                Three separate psum accumulation groups (g / i,f / o) with
                their own stops so tanh(g) and sigmoid(i,f) overlap the tail
                of the matmul block. c-update is one fused tensor_tensor_scan:
                pairs (0, sf) x (c, u) -> c' = sf*c + u."""
                p = off % 2
                src, dst = cpair[p], cpair[1 - p]
                psG = ppa.tile([128, 4], F32, tag="pG", name="pG")
                psIF = ppa.tile([128, 8], F32, tag="pIF", name="pIF")
                psO = ppa.tile([128, 4], F32, tag="pO", name="pO")
                # gx injects (identity matmul), open the accum groups
                nc.tensor.matmul(
                    psG[:, :], identb[:, :], gx_sl[:, 0:4], start=True, stop=False
                )
                nc.tensor.matmul(
                    psIF[:, :], identb[:, :], gx_sl[:, 4:12], start=True, stop=False
                )
                nc.tensor.matmul(
                    psO[:, :], identb[:, :], gx_sl[:, 12:16], start=True, stop=False
                )
                # W_hh matmuls, slot-major (g,g,i,i,f,f,o,o) with per-group stop
                for s in range(8):
                    if s < 2:
                        ps, col0 = psG, s * 2
                    elif s < 6:
                        ps, col0 = psIF, (s - 2) * 2
                    else:
                        ps, col0 = psO, (s - 6) * 2
                    for kc in range(2):
                        rhs = ring[:, off * 4 + kc * 2 : off * 4 + kc * 2 + 2]
                        nc.tensor.matmul(
                            ps[:, col0 : col0 + 2],
                            whh[par][:, (s * 2 + kc) * 128 : (s * 2 + kc + 1) * 128],
                            rhs,
                            start=False,
                            stop=(kc == 1 and s in (1, 5, 7)),
                        )
                # chain: tanh_g + sigmoid(i,f) overlap the MM tail; scan c-update
                gt = gtt[p]
                tch = tcht[p]
                sgf = sigif[p]
                sigo = cp.tile([128, 4], F32, tag="sigo", name="sigo")
                nc.scalar.activation(gt[:, :], psG[:, :], AF.Tanh)
                nc.scalar.activation(
                    _mkap(sgf, 0, [(9, 2), (2, 4)]),
                    _mkap(psIF, 0, [(4, 2), (1, 4)]),
                    AF.Sigmoid,
                )
                nc.scalar.activation(sigo[:, :], psO[:, :], AF.Sigmoid)
                # u = sig_i * tanh_g -> src even cols {2,4,6,8}
                nc.vector.tensor_mul(
                    _mkap(src, 2, [(2, 4)]), _mkap(sgf, 0, [(2, 4)]), gt[:, :]
                )
                # c' = sf*c + u via scan over (0,sf0,0,sf1,..) x (c0,u0,c1,u1,..)
                nc.vector.tensor_tensor_scan(
                    dst[:, 0:8], sgf[:, 8:16], src[:, 1:9], 0.0,
                    ALU.mult, ALU.add,
                )
                nc.scalar.activation(tch[:, :], _mkap(dst, 1, [(2, 4)]), AF.Tanh)
                nc.vector.tensor_mul(
                    ring[:, (off + 1) * 4 : (off + 2) * 4], sigo[:, :], tch[:, :]
                )

            def emit_scan(par):
                # init state
                nc.vector.memset(ring[:, 0:4], 0.0)
                nc.vector.memset(cpair[0][:, :], 0.0)
                nc.vector.memset(cpair[1][:, :], 0.0)
                # prime gx ring slots 0,1 (steps 0..2*SUB)
                for q in range(2):
                    emit_gemm_tile(par, q * SUB, q)
                with tc.For_i(0, NIT, 1) as it:
                    tg = it * U
                    for q in range(NSUB):
                        for tr in range(SUB):
                            off = q * SUB + tr
                            emit_step(par, off, gxr[q % 4][:, tr * 16 : (tr + 1) * 16])
                        emit_gemm_tile(par, tg + (q + 2) * SUB, (q + 2) % 4)
                    # drain h ring to the big y buffer; wrap last h to slot 0
                    t4 = it * (4 * U)
                    nc.sync.dma_start(
                        ybig[:, ds(t4, 4 * U)], ring[:, 4 : 4 * (U + 1)]
                    )
                    nc.vector.tensor_copy(
                        ring[:, 0:4], ring[:, 4 * U : 4 * (U + 1)]
                    )

            def emit_residual(par):
                if par == 0:
                    # x[t'] += y[i*(T/IL)+j] for t' = j*IL + i  (in-place)
                    for hb in range(2):
                        for beta in range(2):
                            seg = hb * 2 + beta
                            xap = _mkap(x32, seg * Tpad, [(IL, T // IL), (1, IL)])
                            xap2 = _mkap(x32, seg * Tpad, [(IL, T // IL), (1, IL)])
                            yap = _mkap(
                                ybig, hb * 2 + beta,
                                [(4, T // IL), (4 * (T // IL), IL)],
                            )
                            nc.vector.tensor_tensor(xap, xap2, yap, ALU.add)
                else:
                    # x_new[t'] = x[T-1-t'] + y[T-1-t']  (flip, via tmp)
                    for hb in range(2):
                        for beta in range(2):
                            seg = hb * 2 + beta
                            yap = _mkap(ybig, hb * 2 + beta, [(4, T)])
                            nc.vector.tensor_tensor(
                                tmpr[:, :],
                                x32[:, seg * Tpad : seg * Tpad + T],
                                yap,
                                ALU.add,
                            )
                            rev = _mkap(tmpr, T - 1, [(-1, T)])
                            nc.vector.tensor_copy(
                                x32[:, seg * Tpad : seg * Tpad + T], rev
                            )
                # refresh bf16 image
                for seg in range(4):
                    nc.vector.tensor_copy(
                        xb[:, seg * Tpad : seg * Tpad + T],
                        x32[:, seg * Tpad : seg * Tpad + T],
                    )

            # ---- layer loop: 2 layers (even, odd) per iteration ----
            with tc.For_i(0, NDL, 1) as lj:
                for par in range(2):
                    lidx = lj * 2 + par
                    nc.sync.dma_start(whh[par][:, :], whh_d[ds(lidx * 128, 128), :])
                    nc.sync.dma_start(wih[par][:, :], wih_d[ds(lidx * 128, 128), :])
                    nc.sync.dma_start(biasb[par][:, :], bias_d[ds(lidx * 128, 128), :])
                    emit_scan(par)
                    emit_residual(par)

            # ---- epilogue: store ----
            for hb in range(2):
                for beta in range(2):
                    seg = hb * 2 + beta
                    nc.sync.dma_start(
                        out_d[beta, hb * 128 : (hb + 1) * 128, :],
                        x32[:, seg * Tpad : seg * Tpad + T],
                    )
    return nc


def prep_weights(w_ih, w_hh, b_ih, b_hh, n_layers, SUB=25):
    """Permute/transpose weights into the SBUF chunk layouts (host side)."""
    whh_all = np.zeros((n_layers * 128, 2048), BF)
    wih_all = np.zeros((n_layers * 128, 2048), BF)
    bias_all = np.zeros((n_layers * 128, 16 * SUB), np.float32)
    # slot order [g_lo,g_hi, i_lo,i_hi, f_lo,f_hi, o_lo,o_hi]; ref gates i,f,g,o
    SLOTS = [(2, 0), (2, 1), (0, 0), (0, 1), (1, 0), (1, 1), (3, 0), (3, 1)]
    for k in range(n_layers):
        bias = (b_ih[k] + b_hh[k]).astype(np.float32)
        for s in range(8):
            g, hf = SLOTS[s]
            r0 = g * C + hf * 128
            rows_hh = w_hh[k][r0 : r0 + 128]  # (128, 256)
            rows_ih = w_ih[k][r0 : r0 + 128]
            for kc in range(2):
                col = (s * 2 + kc) * 128
                whh_all[k * 128 : (k + 1) * 128, col : col + 128] = (
                    rows_hh[:, kc * 128 : (kc + 1) * 128].T.astype(BF)
                )
                wih_all[k * 128 : (k + 1) * 128, col : col + 128] = (
                    rows_ih[:, kc * 128 : (kc + 1) * 128].T.astype(BF)
                )
            # bias layout [m][tr][beta], m == slot
            bb = bias[r0 : r0 + 128]  # (128,)
            bias_all[k * 128 : (k + 1) * 128, s * 2 * SUB : (s + 1) * 2 * SUB] = (
                np.repeat(bb[:, None], 2 * SUB, axis=1)
            )
    return whh_all, wih_all, bias_all


def _timed_pjrt_run(nc, in_maps, n_timing=3):
    """Compile once via PJRT, run repeatedly on the 8 cores, return
    (per-core results, best wall-clock ns per execution)."""
    import time as _time

    import jax
    from jax.sharding import Mesh, PartitionSpec, NamedSharding
    from jax.experimental.shard_map import shard_map

    from concourse import bass2jax, mybir as _mybir

    bass2jax.install_neuronx_cc_hook()
    n_cores = len(in_maps)

    partition_name = nc.partition_id_tensor.name if nc.partition_id_tensor else None
    in_names, out_names, out_avals, zero_outs = [], [], [], []
    for alloc in nc.m.functions[0].allocations:
        if not isinstance(alloc, _mybir.MemoryLocationSet):
            continue
        name = alloc.memorylocations[0].name
        if alloc.kind == "ExternalInput":
            if name != partition_name:
                in_names.append(name)
        elif alloc.kind == "ExternalOutput":
            shape = tuple(alloc.tensor_shape)
            dtype = _mybir.dt.np(alloc.dtype)
            out_names.append(name)
            out_avals.append(jax.core.ShapedArray(shape, dtype))
            zero_outs.append(np.zeros(shape, dtype))
    n_params = len(in_names)
    all_in_names = list(in_names) + list(out_names)
    if partition_name is not None:
        all_in_names.append(partition_name)

    def _body(*args):
        operands = list(args)
        if partition_name is not None:
            operands.append(bass2jax.partition_id_tensor())
        outs = bass2jax._bass_exec_p.bind(
                *operands,
                out_avals=tuple(out_avals),
                in_names=tuple(all_in_names),
                out_names=tuple(out_names),
                lowering_input_output_aliases=(),
                sim_require_finite=True,
                sim_require_nnan=True,
                nc=nc,
            )
        return tuple(outs)

    devices = jax.devices()[:n_cores]
    mesh = Mesh(np.asarray(devices), ("core",))
    nsh = NamedSharding(mesh, PartitionSpec("core"))
    in_specs = (PartitionSpec("core"),) * (n_params + len(out_names))
    out_specs = (PartitionSpec("core"),) * len(out_names)
    sharded = jax.jit(
        shard_map(_body, mesh=mesh, in_specs=in_specs, out_specs=out_specs,
                  check_rep=False),
        keep_unused=True,
    )
    concat_in = [
        np.concatenate([np.asarray(in_maps[c][nm]) for c in range(n_cores)], axis=0)
        for nm in in_names
    ]
    concat_zeros = [
        np.zeros((n_cores * z.shape[0], *z.shape[1:]), z.dtype) for z in zero_outs
    ]
    dev_args = [jax.device_put(a, nsh) for a in concat_in + concat_zeros]
    outs = sharded(*dev_args)
    jax.block_until_ready(outs)
    best = None
    for _ in range(n_timing):
        t0 = _time.perf_counter()
        outs = sharded(*dev_args)
        jax.block_until_ready(outs)
        dt = (_time.perf_counter() - t0) * 1e9
        best = dt if best is None else min(best, dt)
    results = [
        {
            nm: np.asarray(outs[i]).reshape(n_cores, *out_avals[i].shape)[c]
            for i, nm in enumerate(out_names)
        }
        for c in range(n_cores)
    ]
    return results, best


def run(inputs, trace=False, T=None, n_layers=None, SUB=25, NSUB=16, n_timing=3):
    """Build+run with timing; returns (full output, best_exec_ns)."""
    return _kernel_impl(
        inputs["x"], inputs["w_ih"], inputs["w_hh"], inputs["b_ih"],
        inputs["b_hh"], T=T, n_layers=n_layers, SUB=SUB, NSUB=NSUB,
        timed=True, n_timing=n_timing,
    )


def kernel(x, w_ih, w_hh, b_ih, b_hh):
    out, _ = _kernel_impl(x, w_ih, w_hh, b_ih, b_hh, NSUB=16)
    return out


def _kernel_impl(x, w_ih, w_hh, b_ih, b_hh, T=None, n_layers=None, SUB=25,
                 NSUB=4, timed=False, n_timing=3):
    x = np.asarray(x, np.float32)
    w_ih = np.asarray(w_ih, np.float32)
    w_hh = np.asarray(w_hh, np.float32)
    b_ih = np.asarray(b_ih, np.float32)
    b_hh = np.asarray(b_hh, np.float32)
    Bb, Cc, Ll = x.shape
    if T is None:
        T = Ll
    if n_layers is None:
        n_layers = w_ih.shape[0]

    whh_all, wih_all, bias_all = prep_weights(w_ih, w_hh, b_ih, b_hh, n_layers, SUB)
    ident = np.eye(128, dtype=np.float32)

    nc = bacc.Bacc("TRN2", debug=False, target_bir_lowering=False, num_devices=NCORES)
    build_kernel(nc, T=T, n_layers=n_layers, SUB=SUB, NSUB=NSUB)
    nc.finalize()

    in_maps = []
    for core in range(NCORES):
        in_maps.append(
            {
                "x_in": x[core * BPC : (core + 1) * BPC, :, :T].copy(),
                "whh_all": whh_all,
                "wih_all": wih_all,
                "bias_all": bias_all,
                "ident": ident,
            }
        )
    if timed:
        results, best_ns = _timed_pjrt_run(nc, in_maps, n_timing=n_timing)
    else:
        res = run_bass_kernel_spmd(nc, in_maps, core_ids=list(range(NCORES)))
        results, best_ns = res.results, None
    out = np.concatenate([results[c]["out"] for c in range(NCORES)], axis=0)
    return out.astype(np.float32), best_ns


if __name__ == "__main__":
    # tiny smoke test vs golden numpy model
    rng = np.random.default_rng(0)
    T = int(os.environ.get("T", "200"))
    NLY = int(os.environ.get("NLY", "2"))
    SUBv = int(os.environ.get("SUBV", "25"))
    NSUBv = int(os.environ.get("NSUBV", "4"))
    x = rng.standard_normal((B, C, T), dtype=np.float32)
    k = 1.0 / np.sqrt(C)
    w_ih = rng.uniform(-k, k, (NL, 4 * C, C)).astype(np.float32)
    w_hh = rng.uniform(-k, k, (NL, 4 * C, C)).astype(np.float32)
    b_ih = rng.uniform(-k, k, (NL, 4 * C)).astype(np.float32)
    b_hh = rng.uniform(-k, k, (NL, 4 * C)).astype(np.float32)

    got, _ = _kernel_impl(
        x, w_ih[:NLY], w_hh[:NLY], b_ih[:NLY], b_hh[:NLY],
        T=T, n_layers=NLY, SUB=SUBv, NSUB=NSUBv,
    )

    from golden import run_golden

    exp = run_golden(x, w_ih[:NLY], w_hh[:NLY], b_ih[:NLY], b_hh[:NLY], NLY)
    err = np.linalg.norm(got - exp) / np.linalg.norm(exp)
    print(f"T={T} NLY={NLY} rel_l2 vs golden = {err:.3e}")



# revision 18
# speedup vs baseline: 1.1728x; 1.1194x over previous
"""DualPathRNN Trainium2 kernel.

12 sequential LSTM layers (C=256, T=4000) over B=16, data-parallel over batch
across 8 NeuronCores (2 batch elements per core). Everything per-layer runs on
one core:

  - input projection GEMM (W_ih @ x + biases) fused into the scan loop, ~50
    timesteps look-ahead, bf16 weights, fp32 psum; inputs staged via SBUF DMA
    so all matmul access patterns are static
  - the serial 4000-step LSTM scan: per step, W_hh (permuted into 16 bf16
    128x128 chunks) is loaded as the stationary operand (FWL) and multiplied
    against h(t-1) (bf16, N=2) read from a static-slot ring. Gate
    pre-activations accumulate into THREE psum groups (g / i,f / o) with
    separate stop flags so tanh(g) and sigmoid(i,f) start while the rest of
    the matmul block is still issuing; gx is injected per group via identity
    matmuls that run during the previous step's activation tail. The c update
    is a single fused VectorE tensor_tensor_scan over interleaved
    (0,sf)x(c,u) pairs (c' = sf*c + u); parity-alternating tiles keep
    write-after-read deps one step removed. h ring is DMA-copied to the full
    y buffer once per loop iteration.
  - residual + time-permutation (even layers) / time-flip (odd layers) as a
    handful of big strided VectorE ops between layers

Self-contained: hardcodes shapes from the problem spec.
"""
import os
import sys

sys.path.insert(0, "/opt/trn_rl_repo")

import numpy as np
import ml_dtypes

from concourse import bass, bacc, mybir
import concourse.tile as tile
from concourse.bass import ds
from concourse.bass_utils import run_bass_kernel_spmd

F32 = mybir.dt.float32
BF16 = mybir.dt.bfloat16
AF = mybir.ActivationFunctionType
ALU = mybir.AluOpType
ET = mybir.EngineType
BF = ml_dtypes.bfloat16

# Problem constants
C = 256
NL = 12
B = 16
L = 4000
IL = 10
NCORES = 8
BPC = B // NCORES  # 2 batch elements per core

# slot order within a 128-channel half: i, f, o, g ; ref row-gate order i,f,g,o
GMAP = [0, 1, 3, 2]


def _mkap(t, off, dims):
    """Build an AP on tile t's tensor: partition dim from t, free dims given as
    [(stride, count), ...] in elements; off is the element offset (may be a
    ScalarValue expression)."""
    base = t[:, 0:1]
    return bass.AP(
        tensor=base.tensor,
        offset=base.offset + off,
        ap=[list(base.ap[0])] + [[s, n] for (s, n) in dims],
    )


def build_kernel(nc, T=L, n_layers=NL, SUB=25, NSUB=4):
    U = SUB * NSUB
    NIT = T // U
    assert NIT * U == T
    NDL = n_layers // 2
    Tpad = T + 2 * SUB
    b = BPC

    x_in = nc.dram_tensor("x_in", [b, C, T], F32, kind="ExternalInput")
    whh_d = nc.dram_tensor("whh_all", [n_layers * 128, 2048], BF16, kind="ExternalInput")
    wih_d = nc.dram_tensor("wih_all", [n_layers * 128, 2048], BF16, kind="ExternalInput")
    bias_d = nc.dram_tensor("bias_all", [n_layers * 128, 16 * SUB], F32, kind="ExternalInput")
    ident_d = nc.dram_tensor("ident", [128, 128], F32, kind="ExternalInput")
    out_d = nc.dram_tensor("out", [b, C, T], F32, kind="ExternalOutput")

    with tile.TileContext(nc) as tc:
        with (
            tc.tile_pool(name="persist", bufs=1) as pp,
            tc.tile_pool(name="chain", bufs=6) as cp,
            tc.tile_pool(name="stage", bufs=3) as sp,
            tc.tile_pool(name="psStep", bufs=1, space="PSUM") as ppa,
            tc.tile_pool(name="psG", bufs=2, space="PSUM") as ppg,
        ):
            x32 = pp.tile([128, 4 * Tpad], F32, tag="x32")
            xb = pp.tile([128, 4 * Tpad], BF16, tag="xb")
            ybig = pp.tile([128, 4 * T], BF16, tag="ybig")
            ring = pp.tile([128, 4 * (U + 1)], BF16, tag="ring")
            ident = pp.tile([128, 128], F32, tag="ident")
            identb = pp.tile([128, 128], BF16, tag="identb")
            # sigmoid staging (parity pair to keep WAR deps ancient):
            # sigma(i) at cols {0,2,4,6}; cols 8..15 hold (0, sf0, 0, sf1, ..)
            # so [8:16] is the scan's d0 operand
            sigif = [pp.tile([128, 16], F32, tag=f"sigif{q}", name=f"sigif{q}")
                     for q in range(2)]
            # c/u pair tiles (ping-pong per step): c at odd cols {1,3,5,7},
            # u at even cols {2,4,6,8}
            cpair = [pp.tile([128, 9], F32, tag=f"cpair{q}", name=f"cpair{q}")
                     for q in range(2)]
            gtt = [pp.tile([128, 4], F32, tag=f"gt{q}", name=f"gt{q}")
                   for q in range(2)]
            tcht = [pp.tile([128, 4], F32, tag=f"tch{q}", name=f"tch{q}")
                    for q in range(2)]
            tmpr = pp.tile([128, T], F32, tag="tmpr")
            whh = [pp.tile([128, 2048], BF16, tag=f"whh{p}", name=f"whh{p}") for p in range(2)]
            wih = [pp.tile([128, 2048], BF16, tag=f"wih{p}", name=f"wih{p}") for p in range(2)]
            biasb = [pp.tile([128, 16 * SUB], F32, tag=f"bias{p}", name=f"bias{p}") for p in range(2)]
            gxr = [pp.tile([128, 16 * SUB], BF16, tag=f"gxr{q}", name=f"gxr{q}") for q in range(4)]


            # ---- prologue: load inputs, build fp32 + bf16 x images ----
            nc.sync.dma_start(ident[:, :], ident_d[:, :])
            nc.vector.tensor_copy(identb[:, :], ident[:, :])
            nc.vector.memset(sigif[0][:, :], 0.0)
            nc.vector.memset(sigif[1][:, :], 0.0)
            for hb in range(2):
                for beta in range(2):
                    seg = hb * 2 + beta
                    nc.sync.dma_start(
                        x32[:, seg * Tpad : seg * Tpad + T],
                        x_in[beta, hb * 128 : (hb + 1) * 128, :],
                    )
            for seg in range(4):
                nc.vector.memset(x32[:, seg * Tpad + T : (seg + 1) * Tpad], 0.0)
            for seg in range(4):
                nc.vector.tensor_copy(
                    xb[:, seg * Tpad : (seg + 1) * Tpad],
                    x32[:, seg * Tpad : (seg + 1) * Tpad],
                )

            gemm_post = []  # deferred gemm DVE ops, drained one per step

            def emit_gemm_mm(par, tg0, slot):
                """Stage x + run the W_ih matmuls for steps [tg0, tg0+SUB).
                The bias add and psum->gxr reorder copy are DEFERRED into the
                DVE slack of later steps (gemm_post) so they never block the
                next step's u-multiply on the Vector queue."""
                stg = sp.tile([128, 4 * SUB], BF16, tag="stg", name="stg")
                # stage [seg][tau] <- xb[seg*Tpad + tg0 + tau]
                nc.sync.dma_start(
                    stg[:, :], _mkap(xb, tg0, [(Tpad, 4), (1, SUB)])
                )
                psG = ppg.tile([128, 16 * SUB], F32, tag="psG", name="psG")
                # zero the bank on DVE, then accumulate W_ih matmuls with no
                # start flag (keeps the slow fp32 N=400 bias-inject matmul off
                # the PE queue; accumulate-onto-zeros is overwrite-equivalent)
                nc.vector.memset(psG[:, :], 0.0)
                for kc in range(2):
                    rhs = _mkap(stg, kc * 2 * SUB, [(1, SUB), (SUB, 2)])
                    for m in range(8):
                        nc.tensor.matmul(
                            psG[:, m * 2 * SUB : (m + 1) * 2 * SUB],
                            wih[par][:, (m * 2 + kc) * 128 : (m * 2 + kc + 1) * 128],
                            rhs,
                            start=False,
                            stop=(kc == 1 and m == 7),
                        )
                return psG

            def _gemm_bias(par, psG):
                # bias add (contiguous, layouts match)
                nc.vector.tensor_tensor(
                    psG[:, :], psG[:, :], biasb[par][:, :], ALU.add
                )

            def _gemm_copy(psG, slot):
                # reorder copy psum [m][tr][beta] -> ring [tr][slot=m][beta]
                src = _mkap(psG, 0, [(2 * SUB, 8), (2, SUB), (1, 2)])
                dst = _mkap(gxr[slot], 0, [(2, 8), (16, SUB), (1, 2)])
                nc.vector.tensor_copy(dst, src)

            def emit_gemm_tile(par, tg0, slot):
                """Inline (non-deferred) gemm tile — used for priming."""
                psG = emit_gemm_mm(par, tg0, slot)
                _gemm_bias(par, psG)
                _gemm_copy(psG, slot)

            def emit_step(par, off, gx_sl):
                """One LSTM step at in-body offset `off` (static). Reads h(t-1)
                from ring slot off, writes h(t) to ring slot off+1.
                Slot order in psum/gx: [g_lo,g_hi, i_lo,i_hi, f_lo,f_hi, o_lo,o_hi]
                x [beta]; free idx = slot*2+beta.

                Three separate psum accumulation groups (g / i,f / o) with
                their own stops so tanh(g) and sigmoid(i,f) overlap the tail
                of the matmul block. c-update is one fused tensor_tensor_scan:
                pairs (0, sf) x (c, u) -> c' = sf*c + u."""
                p = off % 2
                src, dst = cpair[p], cpair[1 - p]
                psG = ppa.tile([128, 4], F32, tag="pG", name="pG")
                psIF = ppa.tile([128, 8], F32, tag="pIF", name="pIF")
                psO = ppa.tile([128, 4], F32, tag="pO", name="pO")
                # gx injects (identity matmul), open the accum groups
                nc.tensor.matmul(
                    psG[:, :], identb[:, :], gx_sl[:, 0:4], start=True, stop=False
                )
                nc.tensor.matmul(
                    psIF[:, :], identb[:, :], gx_sl[:, 4:12], start=True, stop=False
                )
                nc.tensor.matmul(
                    psO[:, :], identb[:, :], gx_sl[:, 12:16], start=True, stop=False
                )
                # W_hh matmuls, slot-major (g,g,i,i,f,f,o,o) with per-group stop
                for s in range(8):
                    if s < 2:
                        ps, col0 = psG, s * 2
                    elif s < 6:
                        ps, col0 = psIF, (s - 2) * 2
                    else:
                        ps, col0 = psO, (s - 6) * 2
                    for kc in range(2):
                        rhs = ring[:, off * 4 + kc * 2 : off * 4 + kc * 2 + 2]
                        nc.tensor.matmul(
                            ps[:, col0 : col0 + 2],
                            whh[par][:, (s * 2 + kc) * 128 : (s * 2 + kc + 1) * 128],
                            rhs,
                            start=False,
                            stop=(kc == 1 and s in (1, 5, 7)),
                        )
                # chain: tanh_g + sigmoid(i,f) overlap the MM tail; scan c-update
                gt = gtt[p]
                tch = tcht[p]
                sgf = sigif[p]
                sigo = cp.tile([128, 4], F32, tag="sigo", name="sigo")
                nc.scalar.activation(gt[:, :], psG[:, :], AF.Tanh)
                nc.scalar.activation(
                    _mkap(sgf, 0, [(9, 2), (2, 4)]),
                    _mkap(psIF, 0, [(4, 2), (1, 4)]),
                    AF.Sigmoid,
                )
                nc.scalar.activation(sigo[:, :], psO[:, :], AF.Sigmoid)
                # u = sig_i * tanh_g -> src even cols {2,4,6,8}
                nc.vector.tensor_mul(
                    _mkap(src, 2, [(2, 4)]), _mkap(sgf, 0, [(2, 4)]), gt[:, :]
                )
                # c' = sf*c + u via scan over (0,sf0,0,sf1,..) x (c0,u0,c1,u1,..)
                nc.vector.tensor_tensor_scan(
                    dst[:, 0:8], sgf[:, 8:16], src[:, 1:9], 0.0,
                    ALU.mult, ALU.add,
                )
                nc.scalar.activation(tch[:, :], _mkap(dst, 1, [(2, 4)]), AF.Tanh)
                nc.vector.tensor_mul(
                    ring[:, (off + 1) * 4 : (off + 2) * 4], sigo[:, :], tch[:, :]
                )

            def emit_scan(par):
                # init state
                nc.vector.memset(ring[:, 0:4], 0.0)
                nc.vector.memset(cpair[0][:, :], 0.0)
                nc.vector.memset(cpair[1][:, :], 0.0)
                # prime gx ring slots 0,1 (steps 0..2*SUB)
                for q in range(2):
                    emit_gemm_tile(par, q * SUB, q)
                with tc.For_i(0, NIT, 1) as it:
                    tg = it * U
                    for q in range(NSUB):
                        for tr in range(SUB):
                            off = q * SUB + tr
                            emit_step(par, off, gxr[q % 4][:, tr * 16 : (tr + 1) * 16])
                            if gemm_post:
                                gemm_post.pop(0)()
                        psG = emit_gemm_mm(par, tg + (q + 2) * SUB, (q + 2) % 4)
                        gemm_post.append(
                            lambda par=par, psG=psG: _gemm_bias(par, psG)
                        )
                        gemm_post.append(
                            lambda psG=psG, slot=(q + 2) % 4: _gemm_copy(psG, slot)
                        )
                    # flush deferred gemm ops before the body ends
                    while gemm_post:
                        gemm_post.pop(0)()
                    # drain h ring to the big y buffer; wrap last h to slot 0
                    t4 = it * (4 * U)
                    nc.sync.dma_start(
                        ybig[:, ds(t4, 4 * U)], ring[:, 4 : 4 * (U + 1)]
                    )
                    nc.vector.tensor_copy(
                        ring[:, 0:4], ring[:, 4 * U : 4 * (U + 1)]
                    )

            def emit_residual(par):
                if par == 0:
                    # x[t'] += y[i*(T/IL)+j] for t' = j*IL + i  (in-place)
                    for hb in range(2):
                        for beta in range(2):
                            seg = hb * 2 + beta
                            xap = _mkap(x32, seg * Tpad, [(IL, T // IL), (1, IL)])
                            xap2 = _mkap(x32, seg * Tpad, [(IL, T // IL), (1, IL)])
                            yap = _mkap(
                                ybig, hb * 2 + beta,
                                [(4, T // IL), (4 * (T // IL), IL)],
                            )
                            nc.vector.tensor_tensor(xap, xap2, yap, ALU.add)
                else:
                    # x_new[t'] = x[T-1-t'] + y[T-1-t']  (flip, via tmp)
                    for hb in range(2):
                        for beta in range(2):
                            seg = hb * 2 + beta
                            yap = _mkap(ybig, hb * 2 + beta, [(4, T)])
                            nc.vector.tensor_tensor(
                                tmpr[:, :],
                                x32[:, seg * Tpad : seg * Tpad + T],
                                yap,
                                ALU.add,
                            )
                            rev = _mkap(tmpr, T - 1, [(-1, T)])
                            nc.vector.tensor_copy(
                                x32[:, seg * Tpad : seg * Tpad + T], rev
                            )
                # refresh bf16 image
                for seg in range(4):
                    nc.vector.tensor_copy(
                        xb[:, seg * Tpad : seg * Tpad + T],
                        x32[:, seg * Tpad : seg * Tpad + T],
                    )

            # ---- layer loop: 2 layers (even, odd) per iteration ----
            with tc.For_i(0, NDL, 1) as lj:
                for par in range(2):
                    lidx = lj * 2 + par
                    nc.sync.dma_start(whh[par][:, :], whh_d[ds(lidx * 128, 128), :])
                    nc.sync.dma_start(wih[par][:, :], wih_d[ds(lidx * 128, 128), :])
                    nc.sync.dma_start(biasb[par][:, :], bias_d[ds(lidx * 128, 128), :])
                    emit_scan(par)
                    emit_residual(par)

            # ---- epilogue: store ----
            for hb in range(2):
                for beta in range(2):
                    seg = hb * 2 + beta
                    nc.sync.dma_start(
                        out_d[beta, hb * 128 : (hb + 1) * 128, :],
                        x32[:, seg * Tpad : seg * Tpad + T],
                    )
    return nc


def prep_weights(w_ih, w_hh, b_ih, b_hh, n_layers, SUB=25):
    """Permute/transpose weights into the SBUF chunk layouts (host side)."""
    whh_all = np.zeros((n_layers * 128, 2048), BF)
    wih_all = np.zeros((n_layers * 128, 2048), BF)
    bias_all = np.zeros((n_layers * 128, 16 * SUB), np.float32)
    # slot order [g_lo,g_hi, i_lo,i_hi, f_lo,f_hi, o_lo,o_hi]; ref gates i,f,g,o
    SLOTS = [(2, 0), (2, 1), (0, 0), (0, 1), (1, 0), (1, 1), (3, 0), (3, 1)]
    for k in range(n_layers):
        bias = (b_ih[k] + b_hh[k]).astype(np.float32)
        for s in range(8):
            g, hf = SLOTS[s]
            r0 = g * C + hf * 128
            rows_hh = w_hh[k][r0 : r0 + 128]  # (128, 256)
            rows_ih = w_ih[k][r0 : r0 + 128]
            for kc in range(2):
                col = (s * 2 + kc) * 128
                whh_all[k * 128 : (k + 1) * 128, col : col + 128] = (
                    rows_hh[:, kc * 128 : (kc + 1) * 128].T.astype(BF)
                )
                wih_all[k * 128 : (k + 1) * 128, col : col + 128] = (
                    rows_ih[:, kc * 128 : (kc + 1) * 128].T.astype(BF)
                )
            # bias layout [m][tr][beta], m == slot
            bb = bias[r0 : r0 + 128]  # (128,)
            bias_all[k * 128 : (k + 1) * 128, s * 2 * SUB : (s + 1) * 2 * SUB] = (
                np.repeat(bb[:, None], 2 * SUB, axis=1)
            )
    return whh_all, wih_all, bias_all


def _timed_pjrt_run(nc, in_maps, n_timing=3):
    """Compile once via PJRT, run repeatedly on the 8 cores, return
    (per-core results, best wall-clock ns per execution)."""
    import time as _time

    import jax
    from jax.sharding import Mesh, PartitionSpec, NamedSharding
    from jax.experimental.shard_map import shard_map

    from concourse import bass2jax, mybir as _mybir

    bass2jax.install_neuronx_cc_hook()
    n_cores = len(in_maps)

    partition_name = nc.partition_id_tensor.name if nc.partition_id_tensor else None
    in_names, out_names, out_avals, zero_outs = [], [], [], []
    for alloc in nc.m.functions[0].allocations:
        if not isinstance(alloc, _mybir.MemoryLocationSet):
            continue
        name = alloc.memorylocations[0].name
        if alloc.kind == "ExternalInput":
            if name != partition_name:
                in_names.append(name)
        elif alloc.kind == "ExternalOutput":
            shape = tuple(alloc.tensor_shape)
            dtype = _mybir.dt.np(alloc.dtype)
            out_names.append(name)
            out_avals.append(jax.core.ShapedArray(shape, dtype))
            zero_outs.append(np.zeros(shape, dtype))
    n_params = len(in_names)
    all_in_names = list(in_names) + list(out_names)
    if partition_name is not None:
        all_in_names.append(partition_name)

    def _body(*args):
        operands = list(args)
        if partition_name is not None:
            operands.append(bass2jax.partition_id_tensor())
        outs = bass2jax._bass_exec_p.bind(
                *operands,
                out_avals=tuple(out_avals),
                in_names=tuple(all_in_names),
                out_names=tuple(out_names),
                lowering_input_output_aliases=(),
                sim_require_finite=True,
                sim_require_nnan=True,
                nc=nc,
            )
        return tuple(outs)

    devices = jax.devices()[:n_cores]
    mesh = Mesh(np.asarray(devices), ("core",))
    nsh = NamedSharding(mesh, PartitionSpec("core"))
    in_specs = (PartitionSpec("core"),) * (n_params + len(out_names))
    out_specs = (PartitionSpec("core"),) * len(out_names)
    sharded = jax.jit(
        shard_map(_body, mesh=mesh, in_specs=in_specs, out_specs=out_specs,
                  check_rep=False),
        keep_unused=True,
    )
    concat_in = [
        np.concatenate([np.asarray(in_maps[c][nm]) for c in range(n_cores)], axis=0)
        for nm in in_names
    ]
    concat_zeros = [
        np.zeros((n_cores * z.shape[0], *z.shape[1:]), z.dtype) for z in zero_outs
    ]
    dev_args = [jax.device_put(a, nsh) for a in concat_in + concat_zeros]
    outs = sharded(*dev_args)
    jax.block_until_ready(outs)
    best = None
    for _ in range(n_timing):
        t0 = _time.perf_counter()
        outs = sharded(*dev_args)
        jax.block_until_ready(outs)
        dt = (_time.perf_counter() - t0) * 1e9
        best = dt if best is None else min(best, dt)
    results = [
        {
            nm: np.asarray(outs[i]).reshape(n_cores, *out_avals[i].shape)[c]
            for i, nm in enumerate(out_names)
        }
        for c in range(n_cores)
    ]
    return results, best


def run(inputs, trace=False, T=None, n_layers=None, SUB=25, NSUB=32, n_timing=3):
    """Build+run with timing; returns (full output, best_exec_ns)."""
    return _kernel_impl(
        inputs["x"], inputs["w_ih"], inputs["w_hh"], inputs["b_ih"],
        inputs["b_hh"], T=T, n_layers=n_layers, SUB=SUB, NSUB=NSUB,
        timed=True, n_timing=n_timing,
    )


def kernel(x, w_ih, w_hh, b_ih, b_hh):
    out, _ = _kernel_impl(x, w_ih, w_hh, b_ih, b_hh, NSUB=32)
    return out


def _kernel_impl(x, w_ih, w_hh, b_ih, b_hh, T=None, n_layers=None, SUB=25,
                 NSUB=4, timed=False, n_timing=3):
    x = np.asarray(x, np.float32)
    w_ih = np.asarray(w_ih, np.float32)
    w_hh = np.asarray(w_hh, np.float32)
    b_ih = np.asarray(b_ih, np.float32)
    b_hh = np.asarray(b_hh, np.float32)
    Bb, Cc, Ll = x.shape
    if T is None:
        T = Ll
    if n_layers is None:
        n_layers = w_ih.shape[0]

    whh_all, wih_all, bias_all = prep_weights(w_ih, w_hh, b_ih, b_hh, n_layers, SUB)
    ident = np.eye(128, dtype=np.float32)

    nc = bacc.Bacc("TRN2", debug=False, target_bir_lowering=False, num_devices=NCORES)
    build_kernel(nc, T=T, n_layers=n_layers, SUB=SUB, NSUB=NSUB)
    nc.finalize()

    in_maps = []
    for core in range(NCORES):
        in_maps.append(
            {
                "x_in": x[core * BPC : (core + 1) * BPC, :, :T].copy(),
                "whh_all": whh_all,
                "wih_all": wih_all,
                "bias_all": bias_all,
                "ident": ident,
            }
        )
    if timed:
        results, best_ns = _timed_pjrt_run(nc, in_maps, n_timing=n_timing)
    else:
        res = run_bass_kernel_spmd(nc, in_maps, core_ids=list(range(NCORES)))
        results, best_ns = res.results, None
    out = np.concatenate([results[c]["out"] for c in range(NCORES)], axis=0)
    return out.astype(np.float32), best_ns


if __name__ == "__main__":
    # tiny smoke test vs golden numpy model
    rng = np.random.default_rng(0)
    T = int(os.environ.get("T", "200"))
    NLY = int(os.environ.get("NLY", "2"))
    SUBv = int(os.environ.get("SUBV", "25"))
    NSUBv = int(os.environ.get("NSUBV", "4"))
    x = rng.standard_normal((B, C, T), dtype=np.float32)
    k = 1.0 / np.sqrt(C)
    w_ih = rng.uniform(-k, k, (NL, 4 * C, C)).astype(np.float32)
    w_hh = rng.uniform(-k, k, (NL, 4 * C, C)).astype(np.float32)
    b_ih = rng.uniform(-k, k, (NL, 4 * C)).astype(np.float32)
    b_hh = rng.uniform(-k, k, (NL, 4 * C)).astype(np.float32)

    got, _ = _kernel_impl(
        x, w_ih[:NLY], w_hh[:NLY], b_ih[:NLY], b_hh[:NLY],
        T=T, n_layers=NLY, SUB=SUBv, NSUB=NSUBv,
    )

    from golden import run_golden

    exp = run_golden(x, w_ih[:NLY], w_hh[:NLY], b_ih[:NLY], b_hh[:NLY], NLY)
    err = np.linalg.norm(got - exp) / np.linalg.norm(exp)
    print(f"T={T} NLY={NLY} rel_l2 vs golden = {err:.3e}")



# revision 20
# speedup vs baseline: 1.5791x; 1.3464x over previous
"""DualPathRNN Trainium2 kernel.

12 sequential LSTM layers (C=256, T=4000) over B=16, data-parallel over batch
across 8 NeuronCores (2 batch elements per core). Everything per-layer runs on
one core:

  - input projection GEMM (W_ih @ x + biases) fused into the scan loop, ~50
    timesteps look-ahead, bf16 weights, fp32 psum; inputs staged via SBUF DMA
    so all matmul access patterns are static
  - the serial 4000-step LSTM scan: per step, W_hh (permuted into 16 bf16
    128x128 chunks) is loaded as the stationary operand (FWL) and multiplied
    against h(t-1) (bf16, N=2) read from a static-slot ring. Gate
    pre-activations accumulate into THREE psum groups (g / i,f / o) with
    separate stop flags so tanh(g) and sigmoid(i,f) start while the rest of
    the matmul block is still issuing; gx is injected per group via identity
    matmuls that run during the previous step's activation tail. The c update
    is a single fused VectorE tensor_tensor_scan over interleaved
    (0,sf)x(c,u) pairs (c' = sf*c + u); parity-alternating tiles keep
    write-after-read deps one step removed. h ring is DMA-copied to the full
    y buffer once per loop iteration.
  - residual + time-permutation (even layers) / time-flip (odd layers) as a
    handful of big strided VectorE ops between layers

Self-contained: hardcodes shapes from the problem spec.
"""
import os
import sys

sys.path.insert(0, "/opt/trn_rl_repo")

import numpy as np
import ml_dtypes

from concourse import bass, bacc, mybir
import concourse.tile as tile
from concourse.bass import ds
from concourse.bass_utils import run_bass_kernel_spmd

F32 = mybir.dt.float32
BF16 = mybir.dt.bfloat16
AF = mybir.ActivationFunctionType
ALU = mybir.AluOpType
ET = mybir.EngineType
BF = ml_dtypes.bfloat16

# Problem constants
C = 256
NL = 12
B = 16
L = 4000
IL = 10
NCORES = 8
BPC = B // NCORES  # 2 batch elements per core

# slot order within a 128-channel half: i, f, o, g ; ref row-gate order i,f,g,o
GMAP = [0, 1, 3, 2]


def _mkap(t, off, dims):
    """Build an AP on tile t's tensor: partition dim from t, free dims given as
    [(stride, count), ...] in elements; off is the element offset (may be a
    ScalarValue expression)."""
    base = t[:, 0:1]
    return bass.AP(
        tensor=base.tensor,
        offset=base.offset + off,
        ap=[list(base.ap[0])] + [[s, n] for (s, n) in dims],
    )


def build_kernel(nc, T=L, n_layers=NL, SUB=25, NSUB=4):
    U = SUB * NSUB
    NIT = T // U
    assert NIT * U == T
    NDL = n_layers // 2
    Tpad = T + 2 * SUB
    b = BPC

    x_in = nc.dram_tensor("x_in", [b, C, T], F32, kind="ExternalInput")
    whh_d = nc.dram_tensor("whh_all", [n_layers * 128, 2048], BF16, kind="ExternalInput")
    wih_d = nc.dram_tensor("wih_all", [n_layers * 128, 2048], BF16, kind="ExternalInput")
    bias_d = nc.dram_tensor("bias_all", [n_layers * 128, 16 * SUB], F32, kind="ExternalInput")
    ident_d = nc.dram_tensor("ident", [128, 128], F32, kind="ExternalInput")
    out_d = nc.dram_tensor("out", [b, C, T], F32, kind="ExternalOutput")

    with tile.TileContext(nc) as tc:
        with (
            tc.tile_pool(name="persist", bufs=1) as pp,
            tc.tile_pool(name="chain", bufs=6) as cp,
            tc.tile_pool(name="stage", bufs=3) as sp,
            tc.tile_pool(name="psStep", bufs=1, space="PSUM") as ppa,
            tc.tile_pool(name="psG", bufs=2, space="PSUM") as ppg,
        ):
            x32 = pp.tile([128, 4 * Tpad], F32, tag="x32")
            xb = pp.tile([128, 4 * Tpad], BF16, tag="xb")
            ybig = pp.tile([128, 4 * T], BF16, tag="ybig")
            ring = pp.tile([128, 4 * (U + 1)], BF16, tag="ring")
            ident = pp.tile([128, 128], F32, tag="ident")
            identb = pp.tile([128, 128], BF16, tag="identb")
            # sigmoid staging (parity pair to keep WAR deps ancient):
            # sigma(i) at cols {0,2,4,6}; cols 8..15 hold (0, sf0, 0, sf1, ..)
            # so [8:16] is the scan's d0 operand
            sigif = [pp.tile([128, 16], F32, tag=f"sigif{q}", name=f"sigif{q}")
                     for q in range(2)]
            # c/u pair tiles (ping-pong per step): c at odd cols {1,3,5,7},
            # u at even cols {2,4,6,8}
            cpair = [pp.tile([128, 9], F32, tag=f"cpair{q}", name=f"cpair{q}")
                     for q in range(2)]
            gtt = [pp.tile([128, 4], F32, tag=f"gt{q}", name=f"gt{q}")
                   for q in range(2)]
            tcht = [pp.tile([128, 4], F32, tag=f"tch{q}", name=f"tch{q}")
                    for q in range(2)]
            # explicit zero-bias vector for chain activations (avoids the
            # implicit const-pool bias AP and its extra wait condition)
            zb = pp.tile([128, 1], F32, tag="zb")
            tmpr = pp.tile([128, T], F32, tag="tmpr")
            whh = [pp.tile([128, 2048], BF16, tag=f"whh{p}", name=f"whh{p}") for p in range(2)]
            wih = [pp.tile([128, 2048], BF16, tag=f"wih{p}", name=f"wih{p}") for p in range(2)]
            biasb = [pp.tile([128, 16 * SUB], F32, tag=f"bias{p}", name=f"bias{p}") for p in range(2)]
            gxr = [pp.tile([128, 16 * SUB], BF16, tag=f"gxr{q}", name=f"gxr{q}") for q in range(4)]


            # ---- prologue: load inputs, build fp32 + bf16 x images ----
            nc.sync.dma_start(ident[:, :], ident_d[:, :])
            nc.vector.tensor_copy(identb[:, :], ident[:, :])
            nc.vector.memset(zb[:, :], 0.0)
            nc.vector.memset(sigif[0][:, :], 0.0)
            nc.vector.memset(sigif[1][:, :], 0.0)
            for hb in range(2):
                for beta in range(2):
                    seg = hb * 2 + beta
                    nc.sync.dma_start(
                        x32[:, seg * Tpad : seg * Tpad + T],
                        x_in[beta, hb * 128 : (hb + 1) * 128, :],
                    )
            for seg in range(4):
                nc.vector.memset(x32[:, seg * Tpad + T : (seg + 1) * Tpad], 0.0)
            for seg in range(4):
                nc.vector.tensor_copy(
                    xb[:, seg * Tpad : (seg + 1) * Tpad],
                    x32[:, seg * Tpad : (seg + 1) * Tpad],
                )

            gemm_post = []  # deferred gemm DVE ops, drained one per step

            def emit_gemm_mm(par, tg0, slot):
                """Stage x + run the W_ih matmuls for steps [tg0, tg0+SUB).
                The bias add and psum->gxr reorder copy are DEFERRED into the
                DVE slack of later steps (gemm_post) so they never block the
                next step's u-multiply on the Vector queue."""
                stg = sp.tile([128, 4 * SUB], BF16, tag="stg", name="stg")
                # stage [seg][tau] <- xb[seg*Tpad + tg0 + tau]
                nc.sync.dma_start(
                    stg[:, :], _mkap(xb, tg0, [(Tpad, 4), (1, SUB)])
                )
                psG = ppg.tile([128, 16 * SUB], F32, tag="psG", name="psG")
                # zero the bank on DVE, then accumulate W_ih matmuls with no
                # start flag (keeps the slow fp32 N=400 bias-inject matmul off
                # the PE queue; accumulate-onto-zeros is overwrite-equivalent)
                nc.vector.memset(psG[:, :], 0.0)
                for kc in range(2):
                    rhs = _mkap(stg, kc * 2 * SUB, [(1, SUB), (SUB, 2)])
                    for m in range(8):
                        nc.tensor.matmul(
                            psG[:, m * 2 * SUB : (m + 1) * 2 * SUB],
                            wih[par][:, (m * 2 + kc) * 128 : (m * 2 + kc + 1) * 128],
                            rhs,
                            start=False,
                            stop=(kc == 1 and m == 7),
                        )
                return psG

            def _gemm_bias(par, psG):
                # bias add (contiguous, layouts match)
                nc.vector.tensor_tensor(
                    psG[:, :], psG[:, :], biasb[par][:, :], ALU.add
                )

            def _gemm_copy(psG, slot):
                # reorder copy psum [m][tr][beta] -> ring [tr][slot=m][beta]
                src = _mkap(psG, 0, [(2 * SUB, 8), (2, SUB), (1, 2)])
                dst = _mkap(gxr[slot], 0, [(2, 8), (16, SUB), (1, 2)])
                nc.vector.tensor_copy(dst, src)

            def emit_gemm_tile(par, tg0, slot):
                """Inline (non-deferred) gemm tile — used for priming."""
                psG = emit_gemm_mm(par, tg0, slot)
                _gemm_bias(par, psG)
                _gemm_copy(psG, slot)

            def emit_step(par, off, gx_sl):
                """One LSTM step at in-body offset `off` (static). Reads h(t-1)
                from ring slot off, writes h(t) to ring slot off+1.
                Slot order in psum/gx: [g_lo,g_hi, i_lo,i_hi, f_lo,f_hi, o_lo,o_hi]
                x [beta]; free idx = slot*2+beta.

                Three separate psum accumulation groups (g / i,f / o) with
                their own stops so tanh(g) and sigmoid(i,f) overlap the tail
                of the matmul block. c-update is one fused tensor_tensor_scan:
                pairs (0, sf) x (c, u) -> c' = sf*c + u."""
                p = off % 2
                src, dst = cpair[p], cpair[1 - p]
                psG = ppa.tile([128, 4], F32, tag="pG", name="pG")
                psIF = ppa.tile([128, 8], F32, tag="pIF", name="pIF")
                psO = ppa.tile([128, 4], F32, tag="pO", name="pO")
                # gx injects (identity matmul), open the accum groups
                nc.tensor.matmul(
                    psG[:, :], identb[:, :], gx_sl[:, 0:4], start=True, stop=False
                )
                nc.tensor.matmul(
                    psIF[:, :], identb[:, :], gx_sl[:, 4:12], start=True, stop=False
                )
                nc.tensor.matmul(
                    psO[:, :], identb[:, :], gx_sl[:, 12:16], start=True, stop=False
                )
                # W_hh matmuls, slot-major (g,g,i,i,f,f,o,o) with per-group stop
                for s in range(8):
                    if s < 2:
                        ps, col0 = psG, s * 2
                    elif s < 6:
                        ps, col0 = psIF, (s - 2) * 2
                    else:
                        ps, col0 = psO, (s - 6) * 2
                    for kc in range(2):
                        rhs = ring[:, off * 4 + kc * 2 : off * 4 + kc * 2 + 2]
                        nc.tensor.matmul(
                            ps[:, col0 : col0 + 2],
                            whh[par][:, (s * 2 + kc) * 128 : (s * 2 + kc + 1) * 128],
                            rhs,
                            start=False,
                            stop=(kc == 1 and s in (1, 5, 7)),
                        )
                # chain: tanh_g + sigmoid(i,f) overlap the MM tail; scan c-update
                gt = gtt[p]
                tch = tcht[p]
                sgf = sigif[p]
                sigo = cp.tile([128, 4], F32, tag="sigo", name="sigo")
                nc.scalar.activation(gt[:, :], psG[:, :], AF.Tanh, bias=zb[:, :])
                nc.scalar.activation(
                    _mkap(sgf, 0, [(9, 2), (2, 4)]),
                    _mkap(psIF, 0, [(4, 2), (1, 4)]),
                    AF.Sigmoid, bias=zb[:, :],
                )
                nc.scalar.activation(sigo[:, :], psO[:, :], AF.Sigmoid, bias=zb[:, :])
                # u = sig_i * tanh_g -> src even cols {2,4,6,8}
                nc.vector.tensor_mul(
                    _mkap(src, 2, [(2, 4)]), _mkap(sgf, 0, [(2, 4)]), gt[:, :]
                )
                # c' = sf*c + u via scan over (0,sf0,0,sf1,..) x (c0,u0,c1,u1,..)
                nc.vector.tensor_tensor_scan(
                    dst[:, 0:8], sgf[:, 8:16], src[:, 1:9], 0.0,
                    ALU.mult, ALU.add,
                )
                nc.scalar.activation(tch[:, :], _mkap(dst, 1, [(2, 4)]), AF.Tanh, bias=zb[:, :])
                nc.vector.tensor_mul(
                    ring[:, (off + 1) * 4 : (off + 2) * 4], sigo[:, :], tch[:, :]
                )

            def emit_scan(par):
                # init state
                nc.vector.memset(ring[:, 0:4], 0.0)
                nc.vector.memset(cpair[0][:, :], 0.0)
                nc.vector.memset(cpair[1][:, :], 0.0)
                # prime gx ring slots 0,1 (steps 0..2*SUB)
                for q in range(2):
                    emit_gemm_tile(par, q * SUB, q)
                with tc.For_i(0, NIT, 1) as it:
                    tg = it * U
                    for q in range(NSUB):
                        for tr in range(SUB):
                            off = q * SUB + tr
                            emit_step(par, off, gxr[q % 4][:, tr * 16 : (tr + 1) * 16])
                            if gemm_post:
                                gemm_post.pop(0)()
                        psG = emit_gemm_mm(par, tg + (q + 2) * SUB, (q + 2) % 4)
                        gemm_post.append(
                            lambda par=par, psG=psG: _gemm_bias(par, psG)
                        )
                        gemm_post.append(
                            lambda psG=psG, slot=(q + 2) % 4: _gemm_copy(psG, slot)
                        )
                    # flush deferred gemm ops before the body ends
                    while gemm_post:
                        gemm_post.pop(0)()
                    # drain h ring to the big y buffer; wrap last h to slot 0
                    t4 = it * (4 * U)
                    nc.sync.dma_start(
                        ybig[:, ds(t4, 4 * U)], ring[:, 4 : 4 * (U + 1)]
                    )
                    nc.vector.tensor_copy(
                        ring[:, 0:4], ring[:, 4 * U : 4 * (U + 1)]
                    )

            def emit_residual(par):
                if par == 0:
                    # x[t'] += y[i*(T/IL)+j] for t' = j*IL + i  (in-place)
                    for hb in range(2):
                        for beta in range(2):
                            seg = hb * 2 + beta
                            xap = _mkap(x32, seg * Tpad, [(IL, T // IL), (1, IL)])
                            xap2 = _mkap(x32, seg * Tpad, [(IL, T // IL), (1, IL)])
                            yap = _mkap(
                                ybig, hb * 2 + beta,
                                [(4, T // IL), (4 * (T // IL), IL)],
                            )
                            nc.vector.tensor_tensor(xap, xap2, yap, ALU.add)
                else:
                    # x_new[t'] = x[T-1-t'] + y[T-1-t']  (flip, via tmp)
                    for hb in range(2):
                        for beta in range(2):
                            seg = hb * 2 + beta
                            yap = _mkap(ybig, hb * 2 + beta, [(4, T)])
                            nc.vector.tensor_tensor(
                                tmpr[:, :],
                                x32[:, seg * Tpad : seg * Tpad + T],
                                yap,
                                ALU.add,
                            )
                            rev = _mkap(tmpr, T - 1, [(-1, T)])
                            nc.vector.tensor_copy(
                                x32[:, seg * Tpad : seg * Tpad + T], rev
                            )
                # refresh bf16 image
                for seg in range(4):
                    nc.vector.tensor_copy(
                        xb[:, seg * Tpad : seg * Tpad + T],
                        x32[:, seg * Tpad : seg * Tpad + T],
                    )

            # ---- layer loop: 2 layers (even, odd) per iteration ----
            with tc.For_i(0, NDL, 1) as lj:
                for par in range(2):
                    lidx = lj * 2 + par
                    nc.sync.dma_start(whh[par][:, :], whh_d[ds(lidx * 128, 128), :])
                    nc.sync.dma_start(wih[par][:, :], wih_d[ds(lidx * 128, 128), :])
                    nc.sync.dma_start(biasb[par][:, :], bias_d[ds(lidx * 128, 128), :])
                    emit_scan(par)
                    emit_residual(par)

            # ---- epilogue: store ----
            for hb in range(2):
                for beta in range(2):
                    seg = hb * 2 + beta
                    nc.sync.dma_start(
                        out_d[beta, hb * 128 : (hb + 1) * 128, :],
                        x32[:, seg * Tpad : seg * Tpad + T],
                    )
    return nc


def prep_weights(w_ih, w_hh, b_ih, b_hh, n_layers, SUB=25):
    """Permute/transpose weights into the SBUF chunk layouts (host side)."""
    whh_all = np.zeros((n_layers * 128, 2048), BF)
    wih_all = np.zeros((n_layers * 128, 2048), BF)
    bias_all = np.zeros((n_layers * 128, 16 * SUB), np.float32)
    # slot order [g_lo,g_hi, i_lo,i_hi, f_lo,f_hi, o_lo,o_hi]; ref gates i,f,g,o
    SLOTS = [(2, 0), (2, 1), (0, 0), (0, 1), (1, 0), (1, 1), (3, 0), (3, 1)]
    for k in range(n_layers):
        bias = (b_ih[k] + b_hh[k]).astype(np.float32)
        for s in range(8):
            g, hf = SLOTS[s]
            r0 = g * C + hf * 128
            rows_hh = w_hh[k][r0 : r0 + 128]  # (128, 256)
            rows_ih = w_ih[k][r0 : r0 + 128]
            for kc in range(2):
                col = (s * 2 + kc) * 128
                whh_all[k * 128 : (k + 1) * 128, col : col + 128] = (
                    rows_hh[:, kc * 128 : (kc + 1) * 128].T.astype(BF)
                )
                wih_all[k * 128 : (k + 1) * 128, col : col + 128] = (
                    rows_ih[:, kc * 128 : (kc + 1) * 128].T.astype(BF)
                )
            # bias layout [m][tr][beta], m == slot
            bb = bias[r0 : r0 + 128]  # (128,)
            bias_all[k * 128 : (k + 1) * 128, s * 2 * SUB : (s + 1) * 2 * SUB] = (
                np.repeat(bb[:, None], 2 * SUB, axis=1)
            )
    return whh_all, wih_all, bias_all


def _timed_pjrt_run(nc, in_maps, n_timing=3):
    """Compile once via PJRT, run repeatedly on the 8 cores, return
    (per-core results, best wall-clock ns per execution)."""
    import time as _time

    import jax
    from jax.sharding import Mesh, PartitionSpec, NamedSharding
    from jax.experimental.shard_map import shard_map

    from concourse import bass2jax, mybir as _mybir

    bass2jax.install_neuronx_cc_hook()
    n_cores = len(in_maps)

    partition_name = nc.partition_id_tensor.name if nc.partition_id_tensor else None
    in_names, out_names, out_avals, zero_outs = [], [], [], []
    for alloc in nc.m.functions[0].allocations:
        if not isinstance(alloc, _mybir.MemoryLocationSet):
            continue
        name = alloc.memorylocations[0].name
        if alloc.kind == "ExternalInput":
            if name != partition_name:
                in_names.append(name)
        elif alloc.kind == "ExternalOutput":
            shape = tuple(alloc.tensor_shape)
            dtype = _mybir.dt.np(alloc.dtype)
            out_names.append(name)
            out_avals.append(jax.core.ShapedArray(shape, dtype))
            zero_outs.append(np.zeros(shape, dtype))
    n_params = len(in_names)
    all_in_names = list(in_names) + list(out_names)
    if partition_name is not None:
        all_in_names.append(partition_name)

    def _body(*args):
        operands = list(args)
        if partition_name is not None:
            operands.append(bass2jax.partition_id_tensor())
        outs = bass2jax._bass_exec_p.bind(
                *operands,
                out_avals=tuple(out_avals),
                in_names=tuple(all_in_names),
                out_names=tuple(out_names),
                lowering_input_output_aliases=(),
                sim_require_finite=True,
                sim_require_nnan=True,
                nc=nc,
            )
        return tuple(outs)

    devices = jax.devices()[:n_cores]
    mesh = Mesh(np.asarray(devices), ("core",))
    nsh = NamedSharding(mesh, PartitionSpec("core"))
    in_specs = (PartitionSpec("core"),) * (n_params + len(out_names))
    out_specs = (PartitionSpec("core"),) * len(out_names)
    sharded = jax.jit(
        shard_map(_body, mesh=mesh, in_specs=in_specs, out_specs=out_specs,
                  check_rep=False),
        keep_unused=True,
    )
    concat_in = [
        np.concatenate([np.asarray(in_maps[c][nm]) for c in range(n_cores)], axis=0)
        for nm in in_names
    ]
    concat_zeros = [
        np.zeros((n_cores * z.shape[0], *z.shape[1:]), z.dtype) for z in zero_outs
    ]
    dev_args = [jax.device_put(a, nsh) for a in concat_in + concat_zeros]
    outs = sharded(*dev_args)
    jax.block_until_ready(outs)
    best = None
    for _ in range(n_timing):
        t0 = _time.perf_counter()
        outs = sharded(*dev_args)
        jax.block_until_ready(outs)
        dt = (_time.perf_counter() - t0) * 1e9
        best = dt if best is None else min(best, dt)
    results = [
        {
            nm: np.asarray(outs[i]).reshape(n_cores, *out_avals[i].shape)[c]
            for i, nm in enumerate(out_names)
        }
        for c in range(n_cores)
    ]
    return results, best


def run(inputs, trace=False, T=None, n_layers=None, SUB=25, NSUB=32, n_timing=3):
    """Build+run with timing; returns (full output, best_exec_ns)."""
    return _kernel_impl(
        inputs["x"], inputs["w_ih"], inputs["w_hh"], inputs["b_ih"],
        inputs["b_hh"], T=T, n_layers=n_layers, SUB=SUB, NSUB=NSUB,
        timed=True, n_timing=n_timing,
    )


def kernel(x, w_ih, w_hh, b_ih, b_hh):
    out, _ = _kernel_impl(x, w_ih, w_hh, b_ih, b_hh, NSUB=32)
    return out


def _kernel_impl(x, w_ih, w_hh, b_ih, b_hh, T=None, n_layers=None, SUB=25,
                 NSUB=4, timed=False, n_timing=3):
    x = np.asarray(x, np.float32)
    w_ih = np.asarray(w_ih, np.float32)
    w_hh = np.asarray(w_hh, np.float32)
    b_ih = np.asarray(b_ih, np.float32)
    b_hh = np.asarray(b_hh, np.float32)
    Bb, Cc, Ll = x.shape
    if T is None:
        T = Ll
    if n_layers is None:
        n_layers = w_ih.shape[0]

    whh_all, wih_all, bias_all = prep_weights(w_ih, w_hh, b_ih, b_hh, n_layers, SUB)
    ident = np.eye(128, dtype=np.float32)

    nc = bacc.Bacc("TRN2", debug=False, target_bir_lowering=False, num_devices=NCORES)
    build_kernel(nc, T=T, n_layers=n_layers, SUB=SUB, NSUB=NSUB)
    nc.finalize()

    in_maps = []
    for core in range(NCORES):
        in_maps.append(
            {
                "x_in": x[core * BPC : (core + 1) * BPC, :, :T].copy(),
                "whh_all": whh_all,
                "wih_all": wih_all,
                "bias_all": bias_all,
                "ident": ident,
            }
        )
    if timed:
        results, best_ns = _timed_pjrt_run(nc, in_maps, n_timing=n_timing)
    else:
        res = run_bass_kernel_spmd(nc, in_maps, core_ids=list(range(NCORES)))
        results, best_ns = res.results, None
    out = np.concatenate([results[c]["out"] for c in range(NCORES)], axis=0)
    return out.astype(np.float32), best_ns


if __name__ == "__main__":
    # tiny smoke test vs golden numpy model
    rng = np.random.default_rng(0)
    T = int(os.environ.get("T", "200"))
    NLY = int(os.environ.get("NLY", "2"))
    SUBv = int(os.environ.get("SUBV", "25"))
    NSUBv = int(os.environ.get("NSUBV", "4"))
    x = rng.standard_normal((B, C, T), dtype=np.float32)
    k = 1.0 / np.sqrt(C)
    w_ih = rng.uniform(-k, k, (NL, 4 * C, C)).astype(np.float32)
    w_hh = rng.uniform(-k, k, (NL, 4 * C, C)).astype(np.float32)
    b_ih = rng.uniform(-k, k, (NL, 4 * C)).astype(np.float32)
    b_hh = rng.uniform(-k, k, (NL, 4 * C)).astype(np.float32)

    got, _ = _kernel_impl(
        x, w_ih[:NLY], w_hh[:NLY], b_ih[:NLY], b_hh[:NLY],
        T=T, n_layers=NLY, SUB=SUBv, NSUB=NSUBv,
    )

    from golden import run_golden

    exp = run_golden(x, w_ih[:NLY], w_hh[:NLY], b_ih[:NLY], b_hh[:NLY], NLY)
    err = np.linalg.norm(got - exp) / np.linalg.norm(exp)
    print(f"T={T} NLY={NLY} rel_l2 vs golden = {err:.3e}")

